# revision 8
# baseline (speedup 1.0000x reference)
"""Causal self-attention (B=4, T=2048, D=1024, H=16) on 8 trn2 NeuronCores.

Sharding: 2 cores per batch element; each core handles 8 heads
(tensor-parallel head split). Each core computes QKV projections for its
heads, causal flash-style attention, and a partial o_proj over its 512
head-dims, emitted as TWO half partials (head-pairs 0-1 and 2-3). Host
sums the four partial o_proj outputs per batch element.

All matmul operands are bf16; accumulation in fp32 PSUM; softmax
normalization bf16.

Schedule (single interleaved stream, keeps PE dense + HAM warm):
  prologue: W/x DMAs, q/k projection for pair 0 slab 0, v for si 0-3
  pair 0 attention, with per-tj pre-emission of its remaining q/k slabs
    and v chunks
  pairs 1-3 attention with a gated filler queue popped between score
    blocks: q/k projections of later pairs, then o_proj of pairs 0-1
    (valid once both are normalized)
  tail: o_proj of pairs 2-3 (evacuations split DVE/ACT)
Within a tj block, scores run 2 blocks ahead of PV so ACT (exp) streams
back-to-back; the scalar queue carries only exp (+ tail evac copies).
"""
from collections import deque

import numpy as np

import concourse.bass as bass
import concourse.tile as tile
from concourse import bacc, mybir
from concourse.bass_utils import run_bass_kernel_spmd

F32 = mybir.dt.float32
BF16 = mybir.dt.bfloat16

T = 2048          # sequence length
D = 1024          # d_model
HL = 8            # local heads per core
DK = 64           # head dim
NPAIR = 4         # head pairs per core
NTJ = 4           # t blocks of 512
NSI = 16          # s chunks of 128
NDC = 8           # d_model chunks of 128
NSLAB = 4         # x^T slabs of 512 t-columns

_CACHE: dict = {}


def _emit(nc, tc, ctx, ins, outs, uid=0):
    xT, wqkvT, qkb, vb, woT, bo = ins
    (y,) = outs

    persist = ctx.enter_context(tc.tile_pool(name="persist", bufs=1))

    # ---- persistent SBUF regions ----
    qT = persist.tile([128, NPAIR, T], BF16, tag="qT")     # [dk-pair, pair, t]
    kT = persist.tile([128, NPAIR, T], BF16, tag="kT")
    v_aug = persist.tile([128, NSI, HL, DK + 1], BF16, tag="vaug")
    ot = persist.tile([128, NPAIR, T], BF16, tag="ot")     # [d'pair, pair, t]
    tri = persist.tile([128, 128], F32, tag="tri")
    qkb_t = persist.tile([128, 8], F32, tag="qkb")
    vb_t = persist.tile([128, 512], F32, tag="vb")
    bo_t = persist.tile([128, D], F32, tag="bo")
    w_t = persist.tile([128, NDC, 3 * 512], BF16, tag="w")
    wo_t = persist.tile([128, NPAIR, D], BF16, tag="wo")
    # denominators: row = pair*32 + h01*16 + tj*4 + (tcol//128); col = t%128
    stag = persist.tile([128, 128], BF16, tag="stag")
    rstag = persist.tile([128, 128], BF16, tag="rstag")

    # ---- prologue DMAs ----
    xs_pool = ctx.enter_context(tc.tile_pool(name="xs", bufs=NSLAB))
    xT_r = xT.rearrange("(c p) t -> p c t", p=128)

    def load_slab(slab):
        xs = xs_pool.tile([128, NDC, 512], BF16, tag="xs")
        for dc in range(NDC):
            nc.gpsimd.dma_start(
                out=xs[:, dc, :], in_=xT_r[:, dc, slab * 512:(slab + 1) * 512])
        return xs

    xs_all = [load_slab(s) for s in range(NSLAB)]

    # weight chunks ordered so pair-0 q/k cols and the v cols arrive first
    w_src = wqkvT.rearrange("(c p) e -> p c e", p=128)
    for lo, hi in [(0, 128), (512, 640), (1024, 1536), (128, 512), (640, 1024)]:
        nc.sync.dma_start(out=w_t[:, :, lo:hi], in_=w_src[:, :, lo:hi])

    nc.gpsimd.memset(tri[:], 0.0)
    nc.gpsimd.affine_select(
        out=tri[:], in_=tri[:], compare_op=mybir.AluOpType.is_ge,
        fill=-1e30, base=0, pattern=[[1, 128]], channel_multiplier=-1,
    )
    nc.sync.dma_start(out=qkb_t[:], in_=qkb[:])
    vb_src = bass.AP(tensor=vb.tensor, offset=vb.offset, ap=[[0, 128]] + list(vb.ap))
    nc.gpsimd.dma_start(out=vb_t[:], in_=vb_src)
    bo_src = bass.AP(tensor=bo.tensor, offset=bo.offset, ap=[[0, 128]] + list(bo.ap))
    nc.gpsimd.dma_start(out=bo_t[:], in_=bo_src)
    nc.gpsimd.dma_start(out=wo_t[:], in_=woT.rearrange("(c p) e -> p c e", p=128))
    ones_t = persist.tile([128, 128], F32, tag="ones")
    nc.gpsimd.memset(ones_t[:], 1.0)
    nc.vector.tensor_copy(
        v_aug[:, :, :, 64],
        ones_t[:].rearrange("p (a b) -> p a b", a=NSI),
    )

    rec_d = nc.dram_tensor(f"rec_scratch_{uid}", [128, 128], BF16).ap()
    rec_flat = rec_d.rearrange("a b -> (a b)")

    # ---- pools ----
    aux_ps = ctx.enter_context(tc.tile_pool(name="aux", bufs=2, space="PSUM"))
    psS = ctx.enter_context(tc.tile_pool(name="psS", bufs=2, space="PSUM"))
    psPV = ctx.enter_context(tc.tile_pool(name="psPV", bufs=2, space="PSUM"))
    e_pool = ctx.enter_context(tc.tile_pool(name="epool", bufs=4))
    sc_pool = ctx.enter_context(tc.tile_pool(name="scpool", bufs=4))
    ys_pool = ctx.enter_context(tc.tile_pool(name="yspool", bufs=3))
    bct_pool = ctx.enter_context(tc.tile_pool(name="bcpool", bufs=2))

    # ---- projection / o_proj emitters ----
    def qk_item(ec, slab):
        acc = aux_ps.tile([128, 512], F32, tag="aux", name=f"acc_{ec}_{slab}")
        for dc in range(NDC):
            nc.tensor.matmul(
                acc[:], w_t[:, dc, ec * 128:(ec + 1) * 128],
                xs_all[slab][:, dc, :],
                start=(dc == 0), stop=(dc == NDC - 1),
            )
        dst = qT if ec < 4 else kT
        pair_ = ec % 4
        nc.vector.tensor_add(
            dst[:, pair_, slab * 512:(slab + 1) * 512], acc[:],
            qkb_t[:, ec:ec + 1].broadcast_to([128, 512]),
        )

    def v_item(si):
        slab, tsub = si // 4, si % 4
        acc = aux_ps.tile([128, 512], F32, tag="aux", name=f"vacc_{si}")
        for dc in range(NDC):
            nc.tensor.matmul(
                acc[:], xs_all[slab][:, dc, tsub * 128:(tsub + 1) * 128],
                w_t[:, dc, 1024:1536],
                start=(dc == 0), stop=(dc == NDC - 1),
            )
        nc.vector.tensor_add(
            v_aug[:, si, :, 0:64],
            acc[:].rearrange("p (h c) -> p h c", h=HL),
            vb_t[:].rearrange("p (h c) -> p h c", h=HL),
        )

    ys_cur = {}

    def oproj_item(half, tc_, ec, tail=False):
        p0, p1 = 2 * half, 2 * half + 1
        if ec == 0:
            ys_cur[(half, tc_)] = ys_pool.tile([128, 1024], BF16, tag="ys",
                                               name=f"ys_{half}_{tc_}")
        ys = ys_cur[(half, tc_)]
        ps = aux_ps.tile([128, 512], F32, tag="aux",
                         name=f"oacc_{half}_{tc_}_{ec}")
        nc.tensor.matmul(
            ps[:], ot[:, p0, tc_ * 128:(tc_ + 1) * 128],
            wo_t[:, p0, ec * 512:(ec + 1) * 512], start=True, stop=False)
        nc.tensor.matmul(
            ps[:], ot[:, p1, tc_ * 128:(tc_ + 1) * 128],
            wo_t[:, p1, ec * 512:(ec + 1) * 512], start=False, stop=True)
        if half == 0:
            nc.vector.tensor_add(
                ys[:, ec * 512:(ec + 1) * 512], ps[:],
                bo_t[:, ec * 512:(ec + 1) * 512])
        elif tail and (tc_ + ec) % 2 == 0:
            # split tail evacuations between ACT (idle) and DVE
            nc.scalar.copy(out=ys[:, ec * 512:(ec + 1) * 512], in_=ps[:])
        else:
            nc.vector.tensor_copy(ys[:, ec * 512:(ec + 1) * 512], ps[:])
        if ec == 1:
            eng = nc.sync if tc_ % 2 == 0 else nc.gpsimd
            eng.dma_start(out=y[half, tc_ * 128:(tc_ + 1) * 128, :], in_=ys[:])

    # ---- gated filler queue ----
    filler = deque()   # items: (gate, tag, fn); gate = norms required first
    norm_done = [0]

    def pop_filler(n=1):
        for _ in range(n):
            if not filler:
                return
            gate, _tag, fn = filler[0]
            if norm_done[0] < gate:
                return
            filler.popleft()
            fn()

    def drain_tag(tag):
        """Force-emit all queued items up to and including the last `tag`
        item (they are ordered, so everything ahead of them goes too)."""
        while any(t == tag for (_g, t, _f) in filler):
            g, _t, fn = filler.popleft()
            assert norm_done[0] >= g, "gated item ahead of required drain"
            fn()

    # ---- attention ----
    def emit_scores(pair, tj, si):
        r = si - 4 * tj
        off = 128 * r if r >= 0 else 0
        n = 512 - off
        S = psS.tile([128, 2, 512], F32, tag="sco")
        E = e_pool.tile([128, 2, 512], BF16, tag="E")
        nc.tensor.matmul(
            S[:, 0, 0:n],
            kT[0:64, pair, si * 128:(si + 1) * 128],
            qT[0:64, pair, tj * 512 + off:(tj + 1) * 512],
            start=True, stop=True,
        )
        nc.tensor.matmul(
            S[:, 1, 0:n],
            kT[64:128, pair, si * 128:(si + 1) * 128],
            qT[64:128, pair, tj * 512 + off:(tj + 1) * 512],
            start=True, stop=True,
            tile_position=(64, 0),
        )
        if r >= 0:
            nc.vector.tensor_add(S[:, 0, 0:128], S[:, 0, 0:128], tri[:])
            nc.vector.tensor_add(S[:, 1, 0:128], S[:, 1, 0:128], tri[:])
        nc.scalar.activation(
            out=E[:, :, off:512], in_=S[:, :, 0:n],
            func=mybir.ActivationFunctionType.Exp, scale=0.125,
        )
        return E, off

    def attention(pair):
        hA, hB = 2 * pair, 2 * pair + 1
        r0p = pair * 32
        for tj in range(NTJ):
            if pair == 0 and tj > 0:
                qk_item(0, tj)
                qk_item(4, tj)
                for si4 in range(4 * tj, 4 * tj + 4):
                    v_item(si4)
            n_si = 4 * tj + 4
            pvA = psPV.tile([65, 512], F32, tag="pv", name=f"pvA_{pair}_{tj}")
            pvB = psPV.tile([65, 512], F32, tag="pv", name=f"pvB_{pair}_{tj}")

            def emit_pv(si, E, off):
                nc.tensor.matmul(
                    pvA[:, off:512], v_aug[:, si, hA, :], E[:, 0, off:512],
                    start=(si == 0), stop=(si == n_si - 1),
                    skip_group_check=True,
                )
                nc.tensor.matmul(
                    pvB[:, off:512], v_aug[:, si, hB, :], E[:, 1, off:512],
                    start=(si == 0), stop=(si == n_si - 1),
                    skip_group_check=True,
                )

            pend = []
            for si in range(n_si):
                pend.append((si, *emit_scores(pair, tj, si)))
                if len(pend) >= 3:
                    emit_pv(*pend.pop(0))
                    pop_filler()
            while pend:
                emit_pv(*pend.pop(0))
                pop_filler()
            # evacuate tj: O^T rows to ot (partition-shift DMA), denom to stag
            for h01, pv in ((0, pvA), (1, pvB)):
                sc = sc_pool.tile([65, 512], BF16, tag="sc",
                                  name=f"sc_{pair}_{tj}_{h01}")
                nc.vector.tensor_copy(sc[:], pv[:])
                nc.sync.dma_start(
                    out=ot[h01 * 64:(h01 + 1) * 64, pair,
                           tj * 512:(tj + 1) * 512],
                    in_=sc[0:64, :],
                )
                r0 = r0p + h01 * 16 + tj * 4
                nc.gpsimd.dma_start(out=stag[r0:r0 + 4, :], in_=sc[64:65, :])
                pop_filler()
        # normalization chain (overlaps the next pair's attention)
        with nc.allow_low_precision(reason="bf16 softmax denominators"):
            nc.vector.reciprocal(rstag[r0p:r0p + 32, :], stag[r0p:r0p + 32, :])
        nc.sync.dma_start(out=rec_d[r0p:r0p + 32, :], in_=rstag[r0p:r0p + 32, :])
        bct = bct_pool.tile([128, NTJ, 512], BF16, tag="bc")
        for h01 in range(2):
            r0 = r0p + h01 * 16
            src = rec_flat[r0 * 128:(r0 + 16) * 128]
            bsrc = bass.AP(tensor=src.tensor, offset=src.offset,
                           ap=[[0, 64]] + list(src.ap))
            nc.gpsimd.dma_start(
                out=bct[h01 * 64:(h01 + 1) * 64, :, :]
                    .rearrange("p a b -> p (a b)"),
                in_=bsrc,
            )
        nc.vector.tensor_mul(
            ot[:, pair, :], ot[:, pair, :],
            bct[:].rearrange("p a b -> p (a b)"),
        )
        norm_done[0] += 1

    # ---- schedule ----
    for p_ in (1, 2, 3):
        for slab in range(NSLAB):
            filler.append((0, f"qk{p_}", lambda e=p_, s=slab: qk_item(e, s)))
            filler.append((0, f"qk{p_}",
                           lambda e=4 + p_, s=slab: qk_item(e, s)))
    for tc_ in range(16):
        for ec in range(2):
            filler.append((2, "oproj0",
                           lambda t=tc_, e=ec: oproj_item(0, t, e)))
    qk_item(0, 0)
    qk_item(4, 0)
    for si in range(4):
        v_item(si)
    attention(0)
    drain_tag("qk1")
    attention(1)
    drain_tag("qk2")
    attention(2)
    drain_tag("qk3")
    attention(3)
    while filler:
        pop_filler()
    for tc_ in range(16):
        for ec in range(2):
            oproj_item(1, tc_, ec, tail=True)


def _build(nrep: int = 1, unroll: int = 8):
    """nrep=1: single-shot kernel (used by kernel()). nrep>1: benchmark
    build -- a device-side For_i loop of nrep/unroll iterations, each
    containing `unroll` unrolled copies of the kernel body."""
    key = ("nc", nrep, unroll)
    if key in _CACHE:
        return _CACHE[key]
    from contextlib import ExitStack

    nc = bacc.Bacc("TRN2", target_bir_lowering=False, debug=False, num_devices=8)
    xT = nc.dram_tensor("xT", [D, T], BF16, kind="ExternalInput").ap()
    wqkvT = nc.dram_tensor("wqkvT", [D, 3 * 512], BF16, kind="ExternalInput").ap()
    qkb = nc.dram_tensor("qkb", [128, 8], F32, kind="ExternalInput").ap()
    vb = nc.dram_tensor("vb", [512], F32, kind="ExternalInput").ap()
    woT = nc.dram_tensor("woT", [512, D], BF16, kind="ExternalInput").ap()
    bo = nc.dram_tensor("bo", [D], F32, kind="ExternalInput").ap()
    y = nc.dram_tensor("y", [2, T, D], BF16, kind="ExternalOutput").ap()

    with tile.TileContext(nc) as tc:
        if nrep == 1:
            with ExitStack() as ctx:
                _emit(nc, tc, ctx, (xT, wqkvT, qkb, vb, woT, bo), (y,))
        else:
            assert nrep % unroll == 0
            with tc.For_i(0, nrep // unroll):
                for u in range(unroll):
                    with ExitStack() as ctx:
                        _emit(nc, tc, ctx, (xT, wqkvT, qkb, vb, woT, bo), (y,),
                              uid=u)
    nc.compile()
    _CACHE[key] = nc
    return nc


def _shard_inputs(x, Wqkv, bqkv, Wo, bo):
    """Build the 8 per-core input maps (x/Wqkv/Wo cast to bf16)."""
    import ml_dtypes
    bf16 = ml_dtypes.bfloat16
    x = np.ascontiguousarray(np.asarray(x, dtype=np.float32))
    Wqkv = np.asarray(Wqkv, dtype=np.float32)
    bqkv = np.asarray(bqkv, dtype=np.float32)
    Wo = np.asarray(Wo, dtype=np.float32)
    bo = np.asarray(bo, dtype=np.float32)

    in_maps = []
    for core in range(8):
        b, hg = core // 2, core % 2
        heads = hg * 8 + np.arange(8)
        rows = (heads[:, None] * 64 + np.arange(64)[None, :]).ravel()  # 512
        q_rows, k_rows, v_rows = rows, 1024 + rows, 2048 + rows
        in_maps.append({
            "xT": np.ascontiguousarray(x[b].T.astype(bf16)),
            "wqkvT": np.ascontiguousarray(
                Wqkv[np.concatenate([q_rows, k_rows, v_rows])].T.astype(bf16)),
            "qkb": np.ascontiguousarray(
                bqkv[np.concatenate([q_rows, k_rows])].reshape(8, 128).T),
            "vb": np.ascontiguousarray(bqkv[v_rows]),
            "woT": np.ascontiguousarray(Wo[:, rows].T.astype(bf16)),
            "bo": (bo if hg == 0 else np.zeros_like(bo)),
        })
    return in_maps


def _get_runner():
    """Build (once) a cached jitted 8-core runner mirroring
    bass2jax.run_bass_via_pjrt, so repeat calls skip retracing."""
    if "runner" in _CACHE:
        return _CACHE["runner"]
    import jax
    from jax.sharding import Mesh, PartitionSpec, NamedSharding
    from jax.experimental.shard_map import shard_map
    from concourse import bass2jax as b2j
    from concourse import mybir as _mb

    nc = _build()
    b2j.install_neuronx_cc_hook()
    partition_name = nc.partition_id_tensor.name if nc.partition_id_tensor else None

    in_names, out_names, out_avals, zero_shapes = [], [], [], []
    for alloc in nc.m.functions[0].allocations:
        if not isinstance(alloc, _mb.MemoryLocationSet):
            continue
        name = alloc.memorylocations[0].name
        if alloc.kind == "ExternalInput":
            if name != partition_name:
                in_names.append(name)
        elif alloc.kind == "ExternalOutput":
            shape = tuple(alloc.tensor_shape)
            dtype = _mb.dt.np(alloc.dtype)
            out_names.append(name)
            out_avals.append(jax.core.ShapedArray(shape, dtype))
            zero_shapes.append((shape, dtype))
    n_params = len(in_names)
    all_names = list(in_names) + list(out_names)
    if partition_name is not None:
        all_names.append(partition_name)

    def _body(*args):
        operands = list(args)
        if partition_name is not None:
            operands.append(b2j.partition_id_tensor())
        outs = b2j._bass_exec_p.bind(
            *operands,
            out_avals=tuple(out_avals),
            in_names=tuple(all_names),
            out_names=tuple(out_names),
            lowering_input_output_aliases=(),
            sim_require_finite=True,
            sim_require_nnan=True,
            nc=nc,
        )
        return tuple(outs)

    devices = jax.devices()[:8]
    mesh = Mesh(np.asarray(devices), ("core",))
    n_outs = len(out_names)
    sharded = jax.jit(
        shard_map(
            _body, mesh=mesh,
            in_specs=(PartitionSpec("core"),) * (n_params + n_outs),
            out_specs=(PartitionSpec("core"),) * n_outs,
            check_rep=False,
        ),
        donate_argnums=tuple(range(n_params, n_params + n_outs)),
        keep_unused=True,
    )
    runner = {
        "sharded": sharded,
        "in_names": in_names,
        "out_names": out_names,
        "zero_shapes": zero_shapes,
        "out_avals": out_avals,
        "shspec": NamedSharding(mesh, PartitionSpec("core")),
    }
    _CACHE["runner"] = runner
    return runner


def _concat_inputs(in_maps, runner):
    return [
        np.concatenate([in_maps[c][name] for c in range(8)], axis=0)
        for name in runner["in_names"]
    ]


def _fresh_zeros(runner):
    return [np.zeros((8 * s[0], *s[1:]), d) for (s, d) in runner["zero_shapes"]]


def kernel(x, Wqkv, bqkv, Wo, bo):
    runner = _get_runner()
    in_maps = _shard_inputs(x, Wqkv, bqkv, Wo, bo)
    out_arrs = runner["sharded"](*_concat_inputs(in_maps, runner),
                                 *_fresh_zeros(runner))
    yi = runner["out_names"].index("y")
    parts = np.asarray(out_arrs[yi]).astype(np.float32).reshape(8, 2, T, D)
    out = np.empty((4, T, D), dtype=np.float32)
    for b in range(4):
        out[b] = (parts[2 * b, 0] + parts[2 * b, 1]
                  + parts[2 * b + 1, 0] + parts[2 * b + 1, 1])
    return out


# revision 12
# speedup vs baseline: 1.0035x; 1.0035x over previous
"""Causal self-attention (B=4, T=2048, D=1024, H=16) on 8 trn2 NeuronCores.

Sharding: 2 cores per batch element; each core handles 8 heads
(tensor-parallel head split). Each core computes QKV projections for its
heads, causal flash-style attention, and a partial o_proj over its 512
head-dims, emitted as TWO half partials (head-pairs 0-1 and 2-3). Host
sums the four partial o_proj outputs per batch element.

All matmul operands are bf16; accumulation in fp32 PSUM; softmax
normalization bf16.

Schedule (single interleaved stream, keeps PE dense + HAM warm):
  prologue: W/x DMAs, q/k projection for pair 0 slab 0, v for si 0-3
  pair 0 attention, with per-tj pre-emission of its remaining q/k slabs
    and v chunks
  pairs 1-3 attention with a gated filler queue popped between score
    blocks: q/k projections of later pairs, then o_proj of pairs 0-1
    (valid once both are normalized)
  tail: o_proj of pairs 2-3 (evacuations split DVE/ACT)
Within a tj block, scores run 2 blocks ahead of PV so ACT (exp) streams
back-to-back; the scalar queue carries only exp (+ tail evac copies).
"""
from collections import deque

import numpy as np

import concourse.bass as bass
import concourse.tile as tile
from concourse import bacc, mybir
from concourse.bass_utils import run_bass_kernel_spmd

F32 = mybir.dt.float32
BF16 = mybir.dt.bfloat16

T = 2048          # sequence length
D = 1024          # d_model
HL = 8            # local heads per core
DK = 64           # head dim
NPAIR = 4         # head pairs per core
NTJ = 4           # t blocks of 512
NSI = 16          # s chunks of 128
NDC = 8           # d_model chunks of 128
NSLAB = 4         # x^T slabs of 512 t-columns

_CACHE: dict = {}


def _emit(nc, tc, ctx, ins, outs, uid=0):
    xT, wqkvT, qkb, vb, woT, bo = ins
    (y,) = outs

    persist = ctx.enter_context(tc.tile_pool(name="persist", bufs=1))

    # ---- persistent SBUF regions ----
    qT = persist.tile([128, NPAIR, T], BF16, tag="qT")     # [dk-pair, pair, t]
    kT = persist.tile([128, NPAIR, T], BF16, tag="kT")
    v_aug = persist.tile([128, NSI, HL, DK + 1], BF16, tag="vaug")
    ot = persist.tile([128, NPAIR, T], BF16, tag="ot")     # [d'pair, pair, t]
    tri = persist.tile([128, 128], F32, tag="tri")
    qkb_t = persist.tile([128, 8], F32, tag="qkb")
    vb_t = persist.tile([128, 512], F32, tag="vb")
    bo_t = persist.tile([128, D], F32, tag="bo")
    w_t = persist.tile([128, NDC, 3 * 512], BF16, tag="w")
    wo_t = persist.tile([128, NPAIR, D], BF16, tag="wo")
    # denominators: row = pair*32 + h01*16 + tj*4 + (tcol//128); col = t%128
    stag = persist.tile([128, 128], BF16, tag="stag")
    rstag = persist.tile([128, 128], BF16, tag="rstag")

    # ---- prologue DMAs ----
    xs_pool = ctx.enter_context(tc.tile_pool(name="xs", bufs=NSLAB))
    xT_r = xT.rearrange("(c p) t -> p c t", p=128)

    def load_slab(slab):
        xs = xs_pool.tile([128, NDC, 512], BF16, tag="xs")
        for dc in range(NDC):
            nc.gpsimd.dma_start(
                out=xs[:, dc, :], in_=xT_r[:, dc, slab * 512:(slab + 1) * 512])
        return xs

    xs_all = [load_slab(s) for s in range(NSLAB)]

    # weight chunks ordered so pair-0 q/k cols and the v cols arrive first
    w_src = wqkvT.rearrange("(c p) e -> p c e", p=128)
    for lo, hi in [(0, 128), (512, 640), (1024, 1536), (128, 512), (640, 1024)]:
        nc.sync.dma_start(out=w_t[:, :, lo:hi], in_=w_src[:, :, lo:hi])

    nc.gpsimd.memset(tri[:], 0.0)
    nc.gpsimd.affine_select(
        out=tri[:], in_=tri[:], compare_op=mybir.AluOpType.is_ge,
        fill=-1e30, base=0, pattern=[[1, 128]], channel_multiplier=-1,
    )
    nc.sync.dma_start(out=qkb_t[:], in_=qkb[:])
    vb_src = bass.AP(tensor=vb.tensor, offset=vb.offset, ap=[[0, 128]] + list(vb.ap))
    nc.gpsimd.dma_start(out=vb_t[:], in_=vb_src)
    bo_src = bass.AP(tensor=bo.tensor, offset=bo.offset, ap=[[0, 128]] + list(bo.ap))
    nc.gpsimd.dma_start(out=bo_t[:], in_=bo_src)
    nc.gpsimd.dma_start(out=wo_t[:], in_=woT.rearrange("(c p) e -> p c e", p=128))
    ones_t = persist.tile([128, 128], F32, tag="ones")
    nc.gpsimd.memset(ones_t[:], 1.0)
    nc.vector.tensor_copy(
        v_aug[:, :, :, 64],
        ones_t[:].rearrange("p (a b) -> p a b", a=NSI),
    )

    rec_d = nc.dram_tensor(f"rec_scratch_{uid}", [128, 128], BF16).ap()
    rec_flat = rec_d.rearrange("a b -> (a b)")

    # ---- pools ----
    aux_ps = ctx.enter_context(tc.tile_pool(name="aux", bufs=2, space="PSUM"))
    psS = ctx.enter_context(tc.tile_pool(name="psS", bufs=2, space="PSUM"))
    psPV = ctx.enter_context(tc.tile_pool(name="psPV", bufs=2, space="PSUM"))
    e_pool = ctx.enter_context(tc.tile_pool(name="epool", bufs=4))
    sc_pool = ctx.enter_context(tc.tile_pool(name="scpool", bufs=4))
    ys_pool = ctx.enter_context(tc.tile_pool(name="yspool", bufs=3))
    bct_pool = ctx.enter_context(tc.tile_pool(name="bcpool", bufs=2))

    # ---- projection / o_proj emitters ----
    def qk_item(ec, slab):
        acc = aux_ps.tile([128, 512], F32, tag="aux", name=f"acc_{ec}_{slab}")
        for dc in range(NDC):
            nc.tensor.matmul(
                acc[:], w_t[:, dc, ec * 128:(ec + 1) * 128],
                xs_all[slab][:, dc, :],
                start=(dc == 0), stop=(dc == NDC - 1),
            )
        dst = qT if ec < 4 else kT
        pair_ = ec % 4
        nc.vector.tensor_add(
            dst[:, pair_, slab * 512:(slab + 1) * 512], acc[:],
            qkb_t[:, ec:ec + 1].broadcast_to([128, 512]),
        )

    def v_item(si):
        slab, tsub = si // 4, si % 4
        acc = aux_ps.tile([128, 512], F32, tag="aux", name=f"vacc_{si}")
        for dc in range(NDC):
            nc.tensor.matmul(
                acc[:], xs_all[slab][:, dc, tsub * 128:(tsub + 1) * 128],
                w_t[:, dc, 1024:1536],
                start=(dc == 0), stop=(dc == NDC - 1),
            )
        nc.vector.tensor_add(
            v_aug[:, si, :, 0:64],
            acc[:].rearrange("p (h c) -> p h c", h=HL),
            vb_t[:].rearrange("p (h c) -> p h c", h=HL),
        )

    ys_cur = {}

    def oproj_item(half, tc_, ec, tail=False):
        p0, p1 = 2 * half, 2 * half + 1
        if ec == 0:
            ys_cur[(half, tc_)] = ys_pool.tile([128, 1024], BF16, tag="ys",
                                               name=f"ys_{half}_{tc_}")
        ys = ys_cur[(half, tc_)]
        # tail groups alternate between two psum pools for 4-bank cycling
        pool = psPV if (tail and (tc_ + ec) % 2) else aux_ps
        tg = "pv" if (tail and (tc_ + ec) % 2) else "aux"
        ps = pool.tile([128, 512], F32, tag=tg,
                       name=f"oacc_{half}_{tc_}_{ec}")
        nc.tensor.matmul(
            ps[:], ot[:, p0, tc_ * 128:(tc_ + 1) * 128],
            wo_t[:, p0, ec * 512:(ec + 1) * 512], start=True, stop=False)
        nc.tensor.matmul(
            ps[:], ot[:, p1, tc_ * 128:(tc_ + 1) * 128],
            wo_t[:, p1, ec * 512:(ec + 1) * 512], start=False, stop=True)
        if half == 0:
            nc.vector.tensor_add(
                ys[:, ec * 512:(ec + 1) * 512], ps[:],
                bo_t[:, ec * 512:(ec + 1) * 512])
        elif tail and (tc_ + ec) % 2 == 0:
            # split tail evacuations between ACT (idle) and DVE
            nc.scalar.copy(out=ys[:, ec * 512:(ec + 1) * 512], in_=ps[:])
        else:
            nc.vector.tensor_copy(ys[:, ec * 512:(ec + 1) * 512], ps[:])
        if ec == 1:
            if not tail:
                eng = nc.gpsimd
            else:
                eng = nc.sync if tc_ % 2 == 0 else nc.scalar
            eng.dma_start(out=y[half, tc_ * 128:(tc_ + 1) * 128, :], in_=ys[:])

    # ---- gated filler queue ----
    filler = deque()   # items: (gate, tag, fn); gate = norms required first
    norm_done = [0]

    def pop_filler(n=1):
        for _ in range(n):
            if not filler:
                return
            gate, _tag, fn = filler[0]
            if norm_done[0] < gate:
                return
            filler.popleft()
            fn()

    def drain_tag(tag):
        """Force-emit all queued items up to and including the last `tag`
        item (they are ordered, so everything ahead of them goes too)."""
        while any(t == tag for (_g, t, _f) in filler):
            g, _t, fn = filler.popleft()
            assert norm_done[0] >= g, "gated item ahead of required drain"
            fn()

    # ---- attention ----
    def emit_scores(pair, tj, si):
        r = si - 4 * tj
        off = 128 * r if r >= 0 else 0
        n = 512 - off
        S = psS.tile([128, 2, 512], F32, tag="sco")
        E = e_pool.tile([128, 2, 512], BF16, tag="E")
        nc.tensor.matmul(
            S[:, 0, 0:n],
            kT[0:64, pair, si * 128:(si + 1) * 128],
            qT[0:64, pair, tj * 512 + off:(tj + 1) * 512],
            start=True, stop=True,
        )
        nc.tensor.matmul(
            S[:, 1, 0:n],
            kT[64:128, pair, si * 128:(si + 1) * 128],
            qT[64:128, pair, tj * 512 + off:(tj + 1) * 512],
            start=True, stop=True,
            tile_position=(64, 0),
        )
        if r >= 0:
            nc.vector.tensor_add(S[:, 0, 0:128], S[:, 0, 0:128], tri[:])
            nc.vector.tensor_add(S[:, 1, 0:128], S[:, 1, 0:128], tri[:])
        nc.scalar.activation(
            out=E[:, :, off:512], in_=S[:, :, 0:n],
            func=mybir.ActivationFunctionType.Exp, scale=0.125,
        )
        return E, off

    def attention(pair):
        hA, hB = 2 * pair, 2 * pair + 1
        r0p = pair * 32
        # the first blocks of a new pair run before any filler so the PE
        # doesn't head-of-line block on the previous pair's norm chain
        no_pop = [4 if pair > 0 else 0]
        tick = [0]

        def maybe_pop():
            if no_pop[0] > 0:
                no_pop[0] -= 1
                return
            tick[0] += 1
            # pair 0 is already PE-rich: pop at half rate so projection
            # items spill into the leaner pair-1 window
            if pair == 0 and tick[0] % 2 == 0:
                return
            pop_filler()

        scA = sc_pool.tile([65, NTJ, 512], BF16, tag="sc", name=f"scA_{pair}")
        scB = sc_pool.tile([65, NTJ, 512], BF16, tag="sc", name=f"scB_{pair}")
        for tj in range(NTJ):
            if pair == 0 and tj > 0:
                qk_item(0, tj)
                qk_item(4, tj)
                for si4 in range(4 * tj, 4 * tj + 4):
                    v_item(si4)
            n_si = 4 * tj + 4
            pvA = psPV.tile([65, 512], F32, tag="pv", name=f"pvA_{pair}_{tj}")
            pvB = psPV.tile([65, 512], F32, tag="pv", name=f"pvB_{pair}_{tj}")

            def emit_pv(si, E, off):
                nc.tensor.matmul(
                    pvA[:, off:512], v_aug[:, si, hA, :], E[:, 0, off:512],
                    start=(si == 0), stop=(si == n_si - 1),
                    skip_group_check=True,
                )
                nc.tensor.matmul(
                    pvB[:, off:512], v_aug[:, si, hB, :], E[:, 1, off:512],
                    start=(si == 0), stop=(si == n_si - 1),
                    skip_group_check=True,
                )

            pend = []
            for si in range(n_si):
                pend.append((si, *emit_scores(pair, tj, si)))
                if len(pend) >= 3:
                    emit_pv(*pend.pop(0))
                    maybe_pop()
            while pend:
                emit_pv(*pend.pop(0))
                maybe_pop()
            # evacuate tj accumulators into the per-pair staging tiles
            nc.vector.tensor_copy(scA[:, tj, :], pvA[:])
            nc.vector.tensor_copy(scB[:, tj, :], pvB[:])
            maybe_pop()
        # batched O^T rows to ot (partition-shift DMA) + denominators
        for h01, sc in ((0, scA), (1, scB)):
            nc.sync.dma_start(
                out=ot[h01 * 64:(h01 + 1) * 64, pair, :],
                in_=sc[0:64, :, :].rearrange("p a b -> p (a b)"),
            )
            r0 = r0p + h01 * 16
            nc.gpsimd.dma_start(out=stag[r0:r0 + 16, :], in_=sc[64:65, :, :])
        # normalization chain (overlaps the next pair's attention)
        with nc.allow_low_precision(reason="bf16 softmax denominators"):
            nc.vector.reciprocal(rstag[r0p:r0p + 32, :], stag[r0p:r0p + 32, :])
        nc.sync.dma_start(out=rec_d[r0p:r0p + 32, :], in_=rstag[r0p:r0p + 32, :])
        bct = bct_pool.tile([128, NTJ, 512], BF16, tag="bc")
        for h01 in range(2):
            r0 = r0p + h01 * 16
            src = rec_flat[r0 * 128:(r0 + 16) * 128]
            bsrc = bass.AP(tensor=src.tensor, offset=src.offset,
                           ap=[[0, 64]] + list(src.ap))
            nc.gpsimd.dma_start(
                out=bct[h01 * 64:(h01 + 1) * 64, :, :]
                    .rearrange("p a b -> p (a b)"),
                in_=bsrc,
            )
        nc.vector.tensor_mul(
            ot[:, pair, :], ot[:, pair, :],
            bct[:].rearrange("p a b -> p (a b)"),
        )
        norm_done[0] += 1

    # ---- schedule ----
    for p_ in (1, 2, 3):
        for slab in range(NSLAB):
            filler.append((0, f"qk{p_}", lambda e=p_, s=slab: qk_item(e, s)))
            filler.append((0, f"qk{p_}",
                           lambda e=4 + p_, s=slab: qk_item(e, s)))
    for tc_ in range(16):
        for ec in range(2):
            filler.append((2, "oproj0",
                           lambda t=tc_, e=ec: oproj_item(0, t, e)))
    qk_item(0, 0)
    qk_item(4, 0)
    for si in range(4):
        v_item(si)
    attention(0)
    drain_tag("qk1")
    attention(1)
    drain_tag("qk2")
    attention(2)
    drain_tag("qk3")
    attention(3)
    while filler:
        pop_filler()
    for tc_ in range(16):
        for ec in range(2):
            oproj_item(1, tc_, ec, tail=True)


def _build(nrep: int = 1, unroll: int = 8):
    """nrep=1: single-shot kernel (used by kernel()). nrep>1: benchmark
    build -- a device-side For_i loop of nrep/unroll iterations, each
    containing `unroll` unrolled copies of the kernel body."""
    key = ("nc", nrep, unroll)
    if key in _CACHE:
        return _CACHE[key]
    from contextlib import ExitStack

    nc = bacc.Bacc("TRN2", target_bir_lowering=False, debug=False, num_devices=8)
    xT = nc.dram_tensor("xT", [D, T], BF16, kind="ExternalInput").ap()
    wqkvT = nc.dram_tensor("wqkvT", [D, 3 * 512], BF16, kind="ExternalInput").ap()
    qkb = nc.dram_tensor("qkb", [128, 8], F32, kind="ExternalInput").ap()
    vb = nc.dram_tensor("vb", [512], F32, kind="ExternalInput").ap()
    woT = nc.dram_tensor("woT", [512, D], BF16, kind="ExternalInput").ap()
    bo = nc.dram_tensor("bo", [D], F32, kind="ExternalInput").ap()
    y = nc.dram_tensor("y", [2, T, D], BF16, kind="ExternalOutput").ap()

    with tile.TileContext(nc) as tc:
        if nrep == 1:
            with ExitStack() as ctx:
                _emit(nc, tc, ctx, (xT, wqkvT, qkb, vb, woT, bo), (y,))
        else:
            assert nrep % unroll == 0
            with tc.For_i(0, nrep // unroll):
                for u in range(unroll):
                    with ExitStack() as ctx:
                        _emit(nc, tc, ctx, (xT, wqkvT, qkb, vb, woT, bo), (y,),
                              uid=u)
    nc.compile()
    _CACHE[key] = nc
    return nc


def _shard_inputs(x, Wqkv, bqkv, Wo, bo):
    """Build the 8 per-core input maps (x/Wqkv/Wo cast to bf16)."""
    import ml_dtypes
    bf16 = ml_dtypes.bfloat16
    x = np.ascontiguousarray(np.asarray(x, dtype=np.float32))
    Wqkv = np.asarray(Wqkv, dtype=np.float32)
    bqkv = np.asarray(bqkv, dtype=np.float32)
    Wo = np.asarray(Wo, dtype=np.float32)
    bo = np.asarray(bo, dtype=np.float32)

    in_maps = []
    for core in range(8):
        b, hg = core // 2, core % 2
        heads = hg * 8 + np.arange(8)
        rows = (heads[:, None] * 64 + np.arange(64)[None, :]).ravel()  # 512
        q_rows, k_rows, v_rows = rows, 1024 + rows, 2048 + rows
        in_maps.append({
            "xT": np.ascontiguousarray(x[b].T.astype(bf16)),
            "wqkvT": np.ascontiguousarray(
                Wqkv[np.concatenate([q_rows, k_rows, v_rows])].T.astype(bf16)),
            "qkb": np.ascontiguousarray(
                bqkv[np.concatenate([q_rows, k_rows])].reshape(8, 128).T),
            "vb": np.ascontiguousarray(bqkv[v_rows]),
            "woT": np.ascontiguousarray(Wo[:, rows].T.astype(bf16)),
            "bo": (bo if hg == 0 else np.zeros_like(bo)),
        })
    return in_maps


def _get_runner():
    """Build (once) a cached jitted 8-core runner mirroring
    bass2jax.run_bass_via_pjrt, so repeat calls skip retracing."""
    if "runner" in _CACHE:
        return _CACHE["runner"]
    import jax
    from jax.sharding import Mesh, PartitionSpec, NamedSharding
    from jax.experimental.shard_map import shard_map
    from concourse import bass2jax as b2j
    from concourse import mybir as _mb

    nc = _build()
    b2j.install_neuronx_cc_hook()
    partition_name = nc.partition_id_tensor.name if nc.partition_id_tensor else None

    in_names, out_names, out_avals, zero_shapes = [], [], [], []
    for alloc in nc.m.functions[0].allocations:
        if not isinstance(alloc, _mb.MemoryLocationSet):
            continue
        name = alloc.memorylocations[0].name
        if alloc.kind == "ExternalInput":
            if name != partition_name:
                in_names.append(name)
        elif alloc.kind == "ExternalOutput":
            shape = tuple(alloc.tensor_shape)
            dtype = _mb.dt.np(alloc.dtype)
            out_names.append(name)
            out_avals.append(jax.core.ShapedArray(shape, dtype))
            zero_shapes.append((shape, dtype))
    n_params = len(in_names)
    all_names = list(in_names) + list(out_names)
    if partition_name is not None:
        all_names.append(partition_name)

    def _body(*args):
        operands = list(args)
        if partition_name is not None:
            operands.append(b2j.partition_id_tensor())
        outs = b2j._bass_exec_p.bind(
            *operands,
            out_avals=tuple(out_avals),
            in_names=tuple(all_names),
            out_names=tuple(out_names),
            lowering_input_output_aliases=(),
            sim_require_finite=True,
            sim_require_nnan=True,
            nc=nc,
        )
        return tuple(outs)

    devices = jax.devices()[:8]
    mesh = Mesh(np.asarray(devices), ("core",))
    n_outs = len(out_names)
    sharded = jax.jit(
        shard_map(
            _body, mesh=mesh,
            in_specs=(PartitionSpec("core"),) * (n_params + n_outs),
            out_specs=(PartitionSpec("core"),) * n_outs,
            check_rep=False,
        ),
        donate_argnums=tuple(range(n_params, n_params + n_outs)),
        keep_unused=True,
    )
    runner = {
        "sharded": sharded,
        "in_names": in_names,
        "out_names": out_names,
        "zero_shapes": zero_shapes,
        "out_avals": out_avals,
        "shspec": NamedSharding(mesh, PartitionSpec("core")),
    }
    _CACHE["runner"] = runner
    return runner


def _concat_inputs(in_maps, runner):
    return [
        np.concatenate([in_maps[c][name] for c in range(8)], axis=0)
        for name in runner["in_names"]
    ]


def _fresh_zeros(runner):
    return [np.zeros((8 * s[0], *s[1:]), d) for (s, d) in runner["zero_shapes"]]


def kernel(x, Wqkv, bqkv, Wo, bo):
    runner = _get_runner()
    in_maps = _shard_inputs(x, Wqkv, bqkv, Wo, bo)
    out_arrs = runner["sharded"](*_concat_inputs(in_maps, runner),
                                 *_fresh_zeros(runner))
    yi = runner["out_names"].index("y")
    parts = np.asarray(out_arrs[yi]).astype(np.float32).reshape(8, 2, T, D)
    out = np.empty((4, T, D), dtype=np.float32)
    for b in range(4):
        out[b] = (parts[2 * b, 0] + parts[2 * b, 1]
                  + parts[2 * b + 1, 0] + parts[2 * b + 1, 1])
    return out


# revision 13
# speedup vs baseline: 1.1671x; 1.1631x over previous
"""Causal self-attention (B=4, T=2048, D=1024, H=16) on 8 trn2 NeuronCores.

Sharding: 2 cores per batch element; each core handles 8 heads
(tensor-parallel head split). Each core computes QKV projections for its
heads, causal flash-style attention, and a partial o_proj over its 512
head-dims, emitted as TWO half partials (head-pairs 0-1 and 2-3). Host
sums the four partial o_proj outputs per batch element.

All matmul operands are bf16; accumulation in fp32 PSUM; softmax
normalization bf16.

Schedule (single interleaved stream, keeps PE dense + HAM warm):
  prologue: W/x DMAs, q/k projection for pair 0 slab 0, v for si 0-3
  pair 0 attention, with per-tj pre-emission of its remaining q/k slabs
    and v chunks
  pairs 1-3 attention with a gated filler queue popped between score
    blocks: q/k projections of later pairs, then o_proj of pairs 0-1
    (valid once both are normalized)
  tail: o_proj of pairs 2-3 (evacuations split DVE/ACT)
Within a tj block, scores run 2 blocks ahead of PV so ACT (exp) streams
back-to-back; the scalar queue carries only exp (+ tail evac copies).
"""
from collections import deque

import numpy as np

import concourse.bass as bass
import concourse.tile as tile
from concourse import bacc, mybir
from concourse.bass_utils import run_bass_kernel_spmd

F32 = mybir.dt.float32
BF16 = mybir.dt.bfloat16

T = 2048          # sequence length
D = 1024          # d_model
HL = 8            # local heads per core
DK = 64           # head dim
NPAIR = 4         # head pairs per core
NTJ = 4           # t blocks of 512
NSI = 16          # s chunks of 128
NDC = 8           # d_model chunks of 128
NSLAB = 4         # x^T slabs of 512 t-columns

_CACHE: dict = {}


def _emit(nc, tc, ctx, ins, outs, uid=0):
    xT, wqkvT, qkb, vb, woT, bo = ins
    (y,) = outs

    persist = ctx.enter_context(tc.tile_pool(name="persist", bufs=1))

    # ---- persistent SBUF regions ----
    qT = persist.tile([128, NPAIR, T], BF16, tag="qT")     # [dk-pair, pair, t]
    kT = persist.tile([128, NPAIR, T], BF16, tag="kT")
    v_aug = persist.tile([128, NSI, HL, DK + 1], BF16, tag="vaug")
    ot = persist.tile([128, NPAIR, T], BF16, tag="ot")     # [d'pair, pair, t]
    tri = persist.tile([128, 128], F32, tag="tri")
    qkb_t = persist.tile([128, 8], F32, tag="qkb")
    vb_t = persist.tile([128, 512], F32, tag="vb")
    bo_t = persist.tile([128, D], F32, tag="bo")
    w_t = persist.tile([128, NDC, 3 * 512], BF16, tag="w")
    wo_t = persist.tile([128, NPAIR, D], BF16, tag="wo")
    # denominators: row = pair*32 + h01*16 + tj*4 + (tcol//128); col = t%128
    stag = persist.tile([128, 128], BF16, tag="stag")
    rstag = persist.tile([128, 128], BF16, tag="rstag")

    # ---- prologue DMAs ----
    xs_pool = ctx.enter_context(tc.tile_pool(name="xs", bufs=NSLAB))
    xT_r = xT.rearrange("(c p) t -> p c t", p=128)

    def load_slab(slab):
        xs = xs_pool.tile([128, NDC, 512], BF16, tag="xs")
        for dc in range(NDC):
            nc.gpsimd.dma_start(
                out=xs[:, dc, :], in_=xT_r[:, dc, slab * 512:(slab + 1) * 512])
        return xs

    # weight chunks ordered so pair-0 q/k cols and the v cols arrive
    # first; the v chunk rides the gpsimd queue in parallel with sync
    w_src = wqkvT.rearrange("(c p) e -> p c e", p=128)
    for lo, hi in [(0, 128), (512, 640)]:
        nc.sync.dma_start(out=w_t[:, :, lo:hi], in_=w_src[:, :, lo:hi])
    xs_all = [load_slab(0)]
    nc.gpsimd.dma_start(out=w_t[:, :, 1024:1536], in_=w_src[:, :, 1024:1536])
    xs_all += [load_slab(s) for s in range(1, NSLAB)]
    for lo, hi in [(128, 512), (640, 1024)]:
        nc.sync.dma_start(out=w_t[:, :, lo:hi], in_=w_src[:, :, lo:hi])

    nc.gpsimd.memset(tri[:], 0.0)
    nc.gpsimd.affine_select(
        out=tri[:], in_=tri[:], compare_op=mybir.AluOpType.is_ge,
        fill=-1e30, base=0, pattern=[[1, 128]], channel_multiplier=-1,
    )
    nc.sync.dma_start(out=qkb_t[:], in_=qkb[:])
    vb_src = bass.AP(tensor=vb.tensor, offset=vb.offset, ap=[[0, 128]] + list(vb.ap))
    nc.gpsimd.dma_start(out=vb_t[:], in_=vb_src)
    bo_src = bass.AP(tensor=bo.tensor, offset=bo.offset, ap=[[0, 128]] + list(bo.ap))
    nc.gpsimd.dma_start(out=bo_t[:], in_=bo_src)
    nc.gpsimd.dma_start(out=wo_t[:], in_=woT.rearrange("(c p) e -> p c e", p=128))
    ones_t = persist.tile([128, 128], F32, tag="ones")
    nc.gpsimd.memset(ones_t[:], 1.0)
    nc.vector.tensor_copy(
        v_aug[:, :, :, 64],
        ones_t[:].rearrange("p (a b) -> p a b", a=NSI),
    )

    rec_d = nc.dram_tensor(f"rec_scratch_{uid}", [128, 128], BF16).ap()
    rec_flat = rec_d.rearrange("a b -> (a b)")

    # ---- pools ----
    aux_ps = ctx.enter_context(tc.tile_pool(name="aux", bufs=2, space="PSUM"))
    psS = ctx.enter_context(tc.tile_pool(name="psS", bufs=2, space="PSUM"))
    psPV = ctx.enter_context(tc.tile_pool(name="psPV", bufs=2, space="PSUM"))
    e_pool = ctx.enter_context(tc.tile_pool(name="epool", bufs=4))
    sc_pool = ctx.enter_context(tc.tile_pool(name="scpool", bufs=4))
    ys_pool = ctx.enter_context(tc.tile_pool(name="yspool", bufs=3))
    bct_pool = ctx.enter_context(tc.tile_pool(name="bcpool", bufs=2))

    # ---- projection / o_proj emitters ----
    def qk_item(ec, slab):
        acc = aux_ps.tile([128, 512], F32, tag="aux", name=f"acc_{ec}_{slab}")
        for dc in range(NDC):
            nc.tensor.matmul(
                acc[:], w_t[:, dc, ec * 128:(ec + 1) * 128],
                xs_all[slab][:, dc, :],
                start=(dc == 0), stop=(dc == NDC - 1),
            )
        dst = qT if ec < 4 else kT
        pair_ = ec % 4
        nc.vector.tensor_add(
            dst[:, pair_, slab * 512:(slab + 1) * 512], acc[:],
            qkb_t[:, ec:ec + 1].broadcast_to([128, 512]),
        )

    def v_item(si):
        slab, tsub = si // 4, si % 4
        acc = aux_ps.tile([128, 512], F32, tag="aux", name=f"vacc_{si}")
        for dc in range(NDC):
            nc.tensor.matmul(
                acc[:], xs_all[slab][:, dc, tsub * 128:(tsub + 1) * 128],
                w_t[:, dc, 1024:1536],
                start=(dc == 0), stop=(dc == NDC - 1),
            )
        nc.vector.tensor_add(
            v_aug[:, si, :, 0:64],
            acc[:].rearrange("p (h c) -> p h c", h=HL),
            vb_t[:].rearrange("p (h c) -> p h c", h=HL),
        )

    ys_cur = {}

    def oproj_item(half, tc_, ec, tail=False):
        p0, p1 = 2 * half, 2 * half + 1
        if ec == 0:
            ys_cur[(half, tc_)] = ys_pool.tile([128, 1024], BF16, tag="ys",
                                               name=f"ys_{half}_{tc_}")
        ys = ys_cur[(half, tc_)]
        # tail groups alternate between two psum pools for 4-bank cycling
        pool = psPV if (tail and (tc_ + ec) % 2) else aux_ps
        tg = "pv" if (tail and (tc_ + ec) % 2) else "aux"
        ps = pool.tile([128, 512], F32, tag=tg,
                       name=f"oacc_{half}_{tc_}_{ec}")
        nc.tensor.matmul(
            ps[:], ot[:, p0, tc_ * 128:(tc_ + 1) * 128],
            wo_t[:, p0, ec * 512:(ec + 1) * 512], start=True, stop=False)
        nc.tensor.matmul(
            ps[:], ot[:, p1, tc_ * 128:(tc_ + 1) * 128],
            wo_t[:, p1, ec * 512:(ec + 1) * 512], start=False, stop=True)
        if half == 0:
            nc.vector.tensor_add(
                ys[:, ec * 512:(ec + 1) * 512], ps[:],
                bo_t[:, ec * 512:(ec + 1) * 512])
        elif tail and (tc_ + ec) % 2 == 0:
            # split tail evacuations between ACT (idle) and DVE
            nc.scalar.copy(out=ys[:, ec * 512:(ec + 1) * 512], in_=ps[:])
        else:
            nc.vector.tensor_copy(ys[:, ec * 512:(ec + 1) * 512], ps[:])
        if ec == 1:
            eng = nc.gpsimd if not tail else nc.scalar
            eng.dma_start(out=y[half, tc_ * 128:(tc_ + 1) * 128, :], in_=ys[:])

    # ---- gated filler queue ----
    filler = deque()   # items: (gate, tag, fn); gate = norms required first
    norm_done = [0]

    def pop_filler(n=1):
        for _ in range(n):
            if not filler:
                return
            gate, _tag, fn = filler[0]
            if norm_done[0] < gate:
                return
            filler.popleft()
            fn()

    def drain_tag(tag):
        """Force-emit all queued items up to and including the last `tag`
        item (they are ordered, so everything ahead of them goes too)."""
        while any(t == tag for (_g, t, _f) in filler):
            g, _t, fn = filler.popleft()
            assert norm_done[0] >= g, "gated item ahead of required drain"
            fn()

    # ---- attention ----
    def emit_scores(pair, tj, si):
        r = si - 4 * tj
        off = 128 * r if r >= 0 else 0
        n = 512 - off
        S = psS.tile([128, 2, 512], F32, tag="sco")
        E = e_pool.tile([128, 2, 512], BF16, tag="E")
        nc.tensor.matmul(
            S[:, 0, 0:n],
            kT[0:64, pair, si * 128:(si + 1) * 128],
            qT[0:64, pair, tj * 512 + off:(tj + 1) * 512],
            start=True, stop=True,
        )
        nc.tensor.matmul(
            S[:, 1, 0:n],
            kT[64:128, pair, si * 128:(si + 1) * 128],
            qT[64:128, pair, tj * 512 + off:(tj + 1) * 512],
            start=True, stop=True,
            tile_position=(64, 0),
        )
        if r >= 0:
            nc.vector.tensor_add(S[:, 0, 0:128], S[:, 0, 0:128], tri[:])
            nc.vector.tensor_add(S[:, 1, 0:128], S[:, 1, 0:128], tri[:])
        nc.scalar.activation(
            out=E[:, :, off:512], in_=S[:, :, 0:n],
            func=mybir.ActivationFunctionType.Exp, scale=0.125,
        )
        return E, off

    def attention(pair):
        hA, hB = 2 * pair, 2 * pair + 1
        r0p = pair * 32
        # the first blocks of a new pair run before any filler so the PE
        # doesn't head-of-line block on the previous pair's norm chain
        no_pop = [4 if pair > 0 else 0]
        tick = [0]

        def maybe_pop():
            if no_pop[0] > 0:
                no_pop[0] -= 1
                return
            tick[0] += 1
            # pair 0 is already PE-rich: pop at half rate so projection
            # items spill into the leaner pair-1 window
            if pair == 0 and tick[0] % 2 == 0:
                return
            pop_filler()

        scA = sc_pool.tile([65, NTJ, 512], BF16, tag="sc", name=f"scA_{pair}")
        scB = sc_pool.tile([65, NTJ, 512], BF16, tag="sc", name=f"scB_{pair}")
        for tj in range(NTJ):
            if pair == 0 and tj > 0:
                qk_item(0, tj)
                qk_item(4, tj)
                for si4 in range(4 * tj, 4 * tj + 4):
                    v_item(si4)
            n_si = 4 * tj + 4
            pvA = psPV.tile([65, 512], F32, tag="pv", name=f"pvA_{pair}_{tj}")
            pvB = psPV.tile([65, 512], F32, tag="pv", name=f"pvB_{pair}_{tj}")

            def emit_pv(si, E, off):
                nc.tensor.matmul(
                    pvA[:, off:512], v_aug[:, si, hA, :], E[:, 0, off:512],
                    start=(si == 0), stop=(si == n_si - 1),
                    skip_group_check=True,
                )
                nc.tensor.matmul(
                    pvB[:, off:512], v_aug[:, si, hB, :], E[:, 1, off:512],
                    start=(si == 0), stop=(si == n_si - 1),
                    skip_group_check=True,
                )

            pend = []
            for si in range(n_si):
                pend.append((si, *emit_scores(pair, tj, si)))
                if len(pend) >= 3:
                    emit_pv(*pend.pop(0))
                    maybe_pop()
            while pend:
                emit_pv(*pend.pop(0))
                maybe_pop()
            # evacuate tj accumulators into the per-pair staging tiles
            nc.vector.tensor_copy(scA[:, tj, :], pvA[:])
            nc.vector.tensor_copy(scB[:, tj, :], pvB[:])
            for h01, sc in ((0, scA), (1, scB)):
                r0 = r0p + h01 * 16 + tj * 4
                nc.gpsimd.dma_start(out=stag[r0:r0 + 4, :],
                                    in_=sc[64:65, tj, :])
            maybe_pop()
        # batched O^T rows to ot (partition-shift DMA)
        for h01, sc in ((0, scA), (1, scB)):
            nc.sync.dma_start(
                out=ot[h01 * 64:(h01 + 1) * 64, pair, :],
                in_=sc[0:64, :, :].rearrange("p a b -> p (a b)"),
            )
        # normalization chain (overlaps the next pair's attention)
        with nc.allow_low_precision(reason="bf16 softmax denominators"):
            nc.vector.reciprocal(rstag[r0p:r0p + 32, :], stag[r0p:r0p + 32, :])
        nc.sync.dma_start(out=rec_d[r0p:r0p + 32, :], in_=rstag[r0p:r0p + 32, :])
        bct = bct_pool.tile([128, NTJ, 512], BF16, tag="bc")
        for h01 in range(2):
            r0 = r0p + h01 * 16
            src = rec_flat[r0 * 128:(r0 + 16) * 128]
            bsrc = bass.AP(tensor=src.tensor, offset=src.offset,
                           ap=[[0, 64]] + list(src.ap))
            nc.gpsimd.dma_start(
                out=bct[h01 * 64:(h01 + 1) * 64, :, :]
                    .rearrange("p a b -> p (a b)"),
                in_=bsrc,
            )
        nc.vector.tensor_mul(
            ot[:, pair, :], ot[:, pair, :],
            bct[:].rearrange("p a b -> p (a b)"),
        )
        norm_done[0] += 1

    # ---- schedule ----
    for p_ in (1, 2, 3):
        for slab in range(NSLAB):
            filler.append((0, f"qk{p_}", lambda e=p_, s=slab: qk_item(e, s)))
            filler.append((0, f"qk{p_}",
                           lambda e=4 + p_, s=slab: qk_item(e, s)))
    for tc_ in range(16):
        for ec in range(2):
            gate = 3 if tc_ >= 10 else 2   # reserve items for window 3
            filler.append((gate, "oproj0",
                           lambda t=tc_, e=ec: oproj_item(0, t, e)))
    qk_item(0, 0)
    qk_item(4, 0)
    for si in range(4):
        v_item(si)
    attention(0)
    drain_tag("qk1")
    attention(1)
    drain_tag("qk2")
    attention(2)
    drain_tag("qk3")
    attention(3)
    while filler:
        pop_filler()
    for tc_ in range(16):
        for ec in range(2):
            oproj_item(1, tc_, ec, tail=True)


def _build(nrep: int = 1, unroll: int = 8):
    """nrep=1: single-shot kernel (used by kernel()). nrep>1: benchmark
    build -- a device-side For_i loop of nrep/unroll iterations, each
    containing `unroll` unrolled copies of the kernel body."""
    key = ("nc", nrep, unroll)
    if key in _CACHE:
        return _CACHE[key]
    from contextlib import ExitStack

    nc = bacc.Bacc("TRN2", target_bir_lowering=False, debug=False, num_devices=8)
    xT = nc.dram_tensor("xT", [D, T], BF16, kind="ExternalInput").ap()
    wqkvT = nc.dram_tensor("wqkvT", [D, 3 * 512], BF16, kind="ExternalInput").ap()
    qkb = nc.dram_tensor("qkb", [128, 8], F32, kind="ExternalInput").ap()
    vb = nc.dram_tensor("vb", [512], F32, kind="ExternalInput").ap()
    woT = nc.dram_tensor("woT", [512, D], BF16, kind="ExternalInput").ap()
    bo = nc.dram_tensor("bo", [D], F32, kind="ExternalInput").ap()
    y = nc.dram_tensor("y", [2, T, D], BF16, kind="ExternalOutput").ap()

    with tile.TileContext(nc) as tc:
        if nrep == 1:
            with ExitStack() as ctx:
                _emit(nc, tc, ctx, (xT, wqkvT, qkb, vb, woT, bo), (y,))
        else:
            assert nrep % unroll == 0
            with tc.For_i(0, nrep // unroll):
                for u in range(unroll):
                    with ExitStack() as ctx:
                        _emit(nc, tc, ctx, (xT, wqkvT, qkb, vb, woT, bo), (y,),
                              uid=u)
    nc.compile()
    _CACHE[key] = nc
    return nc


def _shard_inputs(x, Wqkv, bqkv, Wo, bo):
    """Build the 8 per-core input maps (x/Wqkv/Wo cast to bf16)."""
    import ml_dtypes
    bf16 = ml_dtypes.bfloat16
    x = np.ascontiguousarray(np.asarray(x, dtype=np.float32))
    Wqkv = np.asarray(Wqkv, dtype=np.float32)
    bqkv = np.asarray(bqkv, dtype=np.float32)
    Wo = np.asarray(Wo, dtype=np.float32)
    bo = np.asarray(bo, dtype=np.float32)

    in_maps = []
    for core in range(8):
        b, hg = core // 2, core % 2
        heads = hg * 8 + np.arange(8)
        rows = (heads[:, None] * 64 + np.arange(64)[None, :]).ravel()  # 512
        q_rows, k_rows, v_rows = rows, 1024 + rows, 2048 + rows
        in_maps.append({
            "xT": np.ascontiguousarray(x[b].T.astype(bf16)),
            "wqkvT": np.ascontiguousarray(
                Wqkv[np.concatenate([q_rows, k_rows, v_rows])].T.astype(bf16)),
            "qkb": np.ascontiguousarray(
                bqkv[np.concatenate([q_rows, k_rows])].reshape(8, 128).T),
            "vb": np.ascontiguousarray(bqkv[v_rows]),
            "woT": np.ascontiguousarray(Wo[:, rows].T.astype(bf16)),
            "bo": (bo if hg == 0 else np.zeros_like(bo)),
        })
    return in_maps


def _get_runner():
    """Build (once) a cached jitted 8-core runner mirroring
    bass2jax.run_bass_via_pjrt, so repeat calls skip retracing."""
    if "runner" in _CACHE:
        return _CACHE["runner"]
    import jax
    from jax.sharding import Mesh, PartitionSpec, NamedSharding
    from jax.experimental.shard_map import shard_map
    from concourse import bass2jax as b2j
    from concourse import mybir as _mb

    nc = _build()
    b2j.install_neuronx_cc_hook()
    partition_name = nc.partition_id_tensor.name if nc.partition_id_tensor else None

    in_names, out_names, out_avals, zero_shapes = [], [], [], []
    for alloc in nc.m.functions[0].allocations:
        if not isinstance(alloc, _mb.MemoryLocationSet):
            continue
        name = alloc.memorylocations[0].name
        if alloc.kind == "ExternalInput":
            if name != partition_name:
                in_names.append(name)
        elif alloc.kind == "ExternalOutput":
            shape = tuple(alloc.tensor_shape)
            dtype = _mb.dt.np(alloc.dtype)
            out_names.append(name)
            out_avals.append(jax.core.ShapedArray(shape, dtype))
            zero_shapes.append((shape, dtype))
    n_params = len(in_names)
    all_names = list(in_names) + list(out_names)
    if partition_name is not None:
        all_names.append(partition_name)

    def _body(*args):
        operands = list(args)
        if partition_name is not None:
            operands.append(b2j.partition_id_tensor())
        outs = b2j._bass_exec_p.bind(
            *operands,
            out_avals=tuple(out_avals),
            in_names=tuple(all_names),
            out_names=tuple(out_names),
            lowering_input_output_aliases=(),
            sim_require_finite=True,
            sim_require_nnan=True,
            nc=nc,
        )
        return tuple(outs)

    devices = jax.devices()[:8]
    mesh = Mesh(np.asarray(devices), ("core",))
    n_outs = len(out_names)
    sharded = jax.jit(
        shard_map(
            _body, mesh=mesh,
            in_specs=(PartitionSpec("core"),) * (n_params + n_outs),
            out_specs=(PartitionSpec("core"),) * n_outs,
            check_rep=False,
        ),
        donate_argnums=tuple(range(n_params, n_params + n_outs)),
        keep_unused=True,
    )
    runner = {
        "sharded": sharded,
        "in_names": in_names,
        "out_names": out_names,
        "zero_shapes": zero_shapes,
        "out_avals": out_avals,
        "shspec": NamedSharding(mesh, PartitionSpec("core")),
    }
    _CACHE["runner"] = runner
    return runner


def _concat_inputs(in_maps, runner):
    return [
        np.concatenate([in_maps[c][name] for c in range(8)], axis=0)
        for name in runner["in_names"]
    ]


def _fresh_zeros(runner):
    return [np.zeros((8 * s[0], *s[1:]), d) for (s, d) in runner["zero_shapes"]]


def kernel(x, Wqkv, bqkv, Wo, bo):
    runner = _get_runner()
    in_maps = _shard_inputs(x, Wqkv, bqkv, Wo, bo)
    out_arrs = runner["sharded"](*_concat_inputs(in_maps, runner),
                                 *_fresh_zeros(runner))
    yi = runner["out_names"].index("y")
    parts = np.asarray(out_arrs[yi]).astype(np.float32).reshape(8, 2, T, D)
    out = np.empty((4, T, D), dtype=np.float32)
    for b in range(4):
        out[b] = (parts[2 * b, 0] + parts[2 * b, 1]
                  + parts[2 * b + 1, 0] + parts[2 * b + 1, 1])
    return out
